# revision 7
# baseline (speedup 1.0000x reference)
"""Multi-head attention (BS=4, SL=2048, D=1024, H=16) on 8 TRN2 NeuronCores.

Sharding: batch x head-half. Core c handles batch c//2, heads (c%2)*8..+8.
Host sums the two partial output projections per batch and adds bo.

Per-core pipeline (all matmuls float32r, 1 PE cycle/row):
  - Host uploads x^T feature-major and weight slices pre-transposed, so
    Q^T/K^T land in [feature, token] layout (exactly what the attention
    matmuls want) and V lands token-major. Q is pre-scaled by
    0.125*log2(e) so softmax exponentials become pure exp2.
  - Scores are computed transposed, S^T[key, query], with the two heads of
    a pair row-packed into the 128x128 PE array (K=64 each) so score
    matmuls run pairwise-concurrent.
  - exp2 runs on the scalar engine (PSUM->SBUF) for most tiles, with a
    slice offloaded to custom DVE poly-and-square ops to balance engines.
  - PV appends a ones column to V so PSUM row 64 accumulates the softmax
    denominator for free; normalization = DVE recip + gpsimd broadcast +
    one fused multiply into the O^T SBUF tile, which is exactly the lhsT
    layout the output projection consumes. No on-device transposes.
  - Emission order interleaves the Q projection halves with the two
    attention query-blocks so ACT/DVE exp work starts ~40% into the
    projection phase.
"""

import numpy as np

BS, SL, D, H, HD = 4, 2048, 1024, 16, 64
NCORES = 8
HPC = H // 2          # heads per core = 8
OF = HPC * HD         # per-core feature slice = 512
KC = D // 128         # 8 contraction chunks of 128
IB = 1024             # query-block width for attention
NIB = SL // IB        # 2
SCALE = 1.0 / np.sqrt(HD)
LOG2E = 1.4426950408889634
QSCALE = SCALE * LOG2E  # folded into Wq/bq on the host
LN2 = 1.0 / LOG2E
VW = OF + HPC         # 520: V with a ones column per head at h*65+64

_CACHE = {}


def _dve_takes(j, head):
    """Which exp tiles run on the DVE custom-op path (engine balancing)."""
    return (head == 1 and j % 4 == 2) or (head == 0 and j == 13)


def _build_nc():
    if "nc" in _CACHE:
        return _CACHE["nc"]
    import concourse.bass as bass  # noqa: F401
    from concourse import bacc
    import concourse.mybir as mybir
    import concourse.tile as tile
    import exp_ops

    f32 = mybir.dt.float32
    f32r = mybir.dt.float32r
    EXP = mybir.ActivationFunctionType.Exp
    ADD = mybir.AluOpType.add

    exp_ops.register()
    nc = bacc.Bacc("TRN2", target_bir_lowering=False, debug=False,
                   num_devices=NCORES)

    xqT = nc.declare_dram_parameter("xqT", [D, SL], f32, isOutput=False)
    xkT = nc.declare_dram_parameter("xkT", [D, SL], f32, isOutput=False)
    xvT = nc.declare_dram_parameter("xvT", [D + 1, SL], f32, isOutput=False)
    wqT = nc.declare_dram_parameter("wqT", [D, OF], f32, isOutput=False)
    wkT = nc.declare_dram_parameter("wkT", [D, OF], f32, isOutput=False)
    wvT = nc.declare_dram_parameter("wvT", [D + 1, OF], f32, isOutput=False)
    bqc = nc.declare_dram_parameter("bqc", [128, 4], f32, isOutput=False)
    bkc = nc.declare_dram_parameter("bkc", [128, 4], f32, isOutput=False)
    ones8 = nc.declare_dram_parameter("ones8", [128, HPC], f32, isOutput=False)
    woT = nc.declare_dram_parameter("woT", [OF, D], f32, isOutput=False)
    out = nc.declare_dram_parameter("out", [SL, D], f32, isOutput=True)

    with tile.TileContext(nc) as tc:
        with tc.tile_pool(name="qkv", bufs=1) as qkv:
            qt = [qkv.tile([128, SL], f32r, name=f"qt{i}", tag=f"qt{i}")
                  for i in range(4)]
            kt = [qkv.tile([128, SL], f32r, name=f"kt{i}", tag=f"kt{i}")
                  for i in range(4)]
            vs = [qkv.tile([128, VW], f32r, name=f"v{i}", tag=f"v{i}")
                  for i in range(16)]

            # ---------------- V projection (bias row; ones cols via DMA) --
            with (
                tc.tile_pool(name="wvp", bufs=1) as wp,
                tc.tile_pool(name="xvp", bufs=2) as xp,
                tc.tile_pool(name="psv", bufs=3, space="PSUM") as psv,
            ):
                w = [wp.tile([128, OF], f32r, name=f"wv{k}", tag=f"wv{k}")
                     for k in range(KC)]
                wb = wp.tile([1, OF], f32r, name="wv8", tag="wv8")
                on8 = wp.tile([128, HPC], f32r, name="on8", tag="on8")
                for k in range(KC):
                    nc.sync.dma_start(
                        out=w[k][:], in_=wvT[k * 128:(k + 1) * 128, :].bitcast(f32r))
                nc.sync.dma_start(out=wb[:], in_=wvT[D:D + 1, :].bitcast(f32r))
                nc.sync.dma_start(out=on8[:], in_=ones8[:].bitcast(f32r))
                for tt in range(16):
                    nc.sync.dma_start(
                        out=vs[tt][:].rearrange("p (h w) -> p h w", w=65)[:, :, 64:65],
                        in_=on8[:].rearrange("p (h w) -> p h w", w=1))
                for tb in range(4):
                    x = [xp.tile([128, 512], f32r, name=f"xv{k}", tag=f"xv{k}")
                         for k in range(KC)]
                    xb = xp.tile([1, 512], f32r, name="xv8", tag="xv8")
                    for k in range(KC):
                        nc.sync.dma_start(
                            out=x[k][:],
                            in_=xvT[k * 128:(k + 1) * 128,
                                    tb * 512:(tb + 1) * 512].bitcast(f32r))
                    nc.sync.dma_start(
                        out=xb[:],
                        in_=xvT[D:D + 1, tb * 512:(tb + 1) * 512].bitcast(f32r))
                    for ts in range(4):
                        tt = tb * 4 + ts
                        pa = psv.tile([128, 512], f32, name="pa", tag="pa")
                        for k in range(KC):
                            nc.tensor.matmul(pa[:], x[k][:, ts * 128:(ts + 1) * 128],
                                             w[k][:], start=(k == 0), stop=False)
                        nc.tensor.matmul(pa[:], xb[:, ts * 128:(ts + 1) * 128],
                                         wb[:], start=False, stop=True)
                        nc.vector.tensor_copy(
                            vs[tt][:].rearrange("p (h w) -> p h w", w=65)[:, :, 0:64],
                            pa[:].rearrange("p (h w) -> p h w", w=64))

            # ---------------- K / Q projections (bias via tensor_scalar) --
            def proj_qk(x_dram, w_tiles, b_tile, dst, tbs, xtag, xbufs, psq):
                with tc.tile_pool(name=f"x_{xtag}{tbs[0]}", bufs=xbufs) as xp:
                    for tb in tbs:
                        x = [xp.tile([128, 512], f32r, name=f"x{xtag}{k}",
                                     tag=f"x{xtag}{k}") for k in range(KC)]
                        for k in range(KC):
                            nc.sync.dma_start(
                                out=x[k][:],
                                in_=x_dram[k * 128:(k + 1) * 128,
                                           tb * 512:(tb + 1) * 512].bitcast(f32r))
                        for of_t in range(4):
                            p = psq.tile([128, 512], f32, name="pq", tag="pq")
                            for k in range(KC):
                                nc.tensor.matmul(
                                    p[:], w_tiles[k][:, of_t * 128:(of_t + 1) * 128],
                                    x[k][:], start=(k == 0), stop=(k == KC - 1))
                            nc.vector.tensor_scalar(
                                out=dst[of_t][:, tb * 512:(tb + 1) * 512],
                                in0=p[:], scalar1=b_tile[:, of_t:of_t + 1],
                                scalar2=None, op0=ADD)

            with (
                tc.tile_pool(name="wkp", bufs=1) as wkp,
                tc.tile_pool(name="psq1", bufs=4, space="PSUM") as psq1,
            ):
                wk = [wkp.tile([128, OF], f32r, name=f"wk{k}", tag=f"wk{k}")
                      for k in range(KC)]
                bk_sb = wkp.tile([128, 4], f32, name="bk_sb", tag="bk_sb")
                for k in range(KC):
                    nc.sync.dma_start(
                        out=wk[k][:], in_=wkT[k * 128:(k + 1) * 128, :].bitcast(f32r))
                nc.sync.dma_start(out=bk_sb[:], in_=bkc[:])
                proj_qk(xkT, wk, bk_sb, kt, (0, 1, 2, 3), "k", 2, psq1)

            with (
                tc.tile_pool(name="wqp", bufs=1) as wqp,
                tc.tile_pool(name="psq2", bufs=4, space="PSUM") as psq2,
            ):
                wq = [wqp.tile([128, OF], f32r, name=f"wq{k}", tag=f"wq{k}")
                      for k in range(KC)]
                bq_sb = wqp.tile([128, 4], f32, name="bq_sb", tag="bq_sb")
                for k in range(KC):
                    nc.sync.dma_start(
                        out=wq[k][:],
                        in_=wqT[k * 128:(k + 1) * 128, :].bitcast(f32r))
                nc.sync.dma_start(out=bq_sb[:], in_=bqc[:])
                proj_qk(xqT, wq, bq_sb, qt, (0, 1, 2, 3), "q", 2, psq2)

            otp_cm = tc.tile_pool(name="otp", bufs=1)
            otp = otp_cm.__enter__()
            ot = [otp.tile([128, SL], f32r, name=f"ot{i}", tag=f"ot{i}")
                  for i in range(4)]
            att_cm = tc.tile_pool(name="att_sb", bufs=1)
            att = att_cm.__enter__()
            es1 = att.tile([128, IB], f32, name="es1", tag="es1")
            es2 = att.tile([128, IB], f32, name="es2", tag="es2")

            def attention_block(ib):
                isl = slice(ib * IB, (ib + 1) * IB)
                with tc.tile_pool(name=f"psa{ib}", bufs=1, space="PSUM") as psa:
                    for hp in range(4):
                        hA, hB = 2 * hp, 2 * hp + 1
                        oA = psa.tile([65, IB], f32, name="oA", tag="oA")
                        oB = psa.tile([65, IB], f32, name="oB", tag="oB")
                        for j in range(16):
                            jsl = slice(j * 128, (j + 1) * 128)
                            sA = psa.tile([128, IB], f32, name="sA", tag="sA")
                            sB = psa.tile([128, IB], f32, name="sB", tag="sB")
                            for nb in range(IB // 512):
                                nsl = slice(nb * 512, (nb + 1) * 512)
                                qsl = slice(ib * IB + nb * 512,
                                            ib * IB + nb * 512 + 512)
                                nc.tensor.matmul(
                                    sA[:, nsl], kt[hp][0:64, jsl],
                                    qt[hp][0:64, qsl],
                                    start=True, stop=True, tile_position=(0, 0))
                                nc.tensor.matmul(
                                    sB[:, nsl], kt[hp][64:128, jsl],
                                    qt[hp][64:128, qsl],
                                    start=True, stop=True, tile_position=(64, 0))
                            ps_ab = (sA, sB)
                            pts = []
                            for head in (0, 1):
                                p = att.tile([128, IB], f32r,
                                             name=f"pt{ib}{hp}{j}{head}",
                                             tag="pt", bufs=3)
                                if _dve_takes(j, head):
                                    exp_ops.emit_exp2(nc, p[:], ps_ab[head][:],
                                                      es1[:], es2[:])
                                else:
                                    nc.scalar.activation(p[:], ps_ab[head][:],
                                                         EXP, scale=float(LN2))
                                pts.append(p)
                            vA = vs[j][:, hA * 65:hA * 65 + 65]
                            vB = vs[j][:, hB * 65:hB * 65 + 65]
                            for nb in range(IB // 512):
                                nsl = slice(nb * 512, (nb + 1) * 512)
                                nc.tensor.matmul(oA[:, nsl], vA, pts[0][:, nsl],
                                                 start=(j == 0), stop=(j == 15))
                                nc.tensor.matmul(oB[:, nsl], vB, pts[1][:, nsl],
                                                 start=(j == 0), stop=(j == 15))
                        for head, o_ps in ((0, oA), (1, oB)):
                            r0 = att.tile([1, IB], f32, name=f"r0{head}",
                                          tag="r0", bufs=2)
                            r1 = att.tile([1, IB], f32, name=f"r1{head}",
                                          tag="r1", bufs=2)
                            bc = att.tile([64, IB], f32, name=f"bc{head}",
                                          tag=f"bc{head}")
                            nc.vector.tensor_copy(r0[:], o_ps[64:65, :])
                            nc.vector.reciprocal_approx_fast(r1[:], r0[:])
                            nc.gpsimd.partition_broadcast(bc[:], r1[0:1, :])
                            nc.vector.tensor_mul(
                                ot[hp][head * 64:head * 64 + 64, isl],
                                o_ps[0:64, :], bc[:])

            attention_block(0)
            attention_block(1)

            att_cm.__exit__(None, None, None)

            # ---------------- output projection ------------------------
            with (
                tc.tile_pool(name="wo", bufs=1) as wop,
                tc.tile_pool(name="ob", bufs=4) as obp,
                tc.tile_pool(name="ps_o", bufs=3, space="PSUM") as pso,
            ):
                wo = [wop.tile([128, D], f32r, name=f"wo{i}", tag=f"wo{i}")
                      for i in range(4)]
                for oc in range(4):
                    nc.sync.dma_start(
                        out=wo[oc][:],
                        in_=woT[oc * 128:(oc + 1) * 128, :].bitcast(f32r))
                for tt in range(16):
                    tsl = slice(tt * 128, (tt + 1) * 128)
                    for nb in range(2):
                        nsl = slice(nb * 512, (nb + 1) * 512)
                        p = pso.tile([128, 512], f32, name="po", tag="po")
                        for oc in range(4):
                            nc.tensor.matmul(p[:], ot[oc][:, tsl], wo[oc][:, nsl],
                                             start=(oc == 0), stop=(oc == 3))
                        ob = obp.tile([128, 512], f32, name="ob", tag="ob")
                        nc.scalar.copy(ob[:], p[:])
                        nc.sync.dma_start(out=out[tsl, nsl], in_=ob[:])
            otp_cm.__exit__(None, None, None)

    nc.compile()
    _CACHE["nc"] = nc
    return nc


def _host_prep(value, key_in, query, Wq, bq, Wk, bk, Wv, bv, Wo, bo):
    f32 = np.float32

    def xT(x_b):
        return np.ascontiguousarray(np.asarray(x_b, f32).T)

    def xT_ones(x_b):
        r = np.empty((D + 1, SL), f32)
        r[:D] = np.asarray(x_b, f32).T
        r[D] = 1.0
        return r

    def wT(W, g, scale=1.0):
        sl = slice(g * OF, (g + 1) * OF)
        return np.ascontiguousarray(
            (np.asarray(W, f32)[sl, :].T * f32(scale)).astype(f32))

    def wT_bias(W, b, g):  # V: weights + bias row
        sl = slice(g * OF, (g + 1) * OF)
        r = np.empty((D + 1, OF), f32)
        r[:D] = np.asarray(W, f32)[sl, :].T
        r[D] = np.asarray(b, f32)[sl]
        return r

    def bcol(b, g, scale=1.0):
        return np.ascontiguousarray(
            (np.asarray(b, f32)[g * OF:(g + 1) * OF] * f32(scale))
            .reshape(4, 128).T.astype(f32))

    xq = [xT(query[b]) for b in range(BS)]
    xk = [xT(key_in[b]) for b in range(BS)]
    xv = [xT_ones(value[b]) for b in range(BS)]
    wq = [wT(Wq, g, QSCALE) for g in range(2)]
    wk = [wT(Wk, g) for g in range(2)]
    wv = [wT_bias(Wv, bv, g) for g in range(2)]
    bqs = [bcol(bq, g, QSCALE) for g in range(2)]
    bks = [bcol(bk, g) for g in range(2)]
    WoT = np.ascontiguousarray(np.asarray(Wo, f32).T)
    wo = [np.ascontiguousarray(WoT[g * OF:(g + 1) * OF, :]) for g in range(2)]
    on8 = np.ones((128, HPC), f32)

    in_maps = []
    for c in range(NCORES):
        b, g = c // 2, c % 2
        in_maps.append({
            "xqT": xq[b], "xkT": xk[b], "xvT": xv[b],
            "wqT": wq[g], "wkT": wk[g], "wvT": wv[g], "woT": wo[g],
            "bqc": bqs[g], "bkc": bks[g], "ones8": on8,
        })
    return in_maps


LAST_EXEC_NS = None
LAST_RESULTS = None


def kernel(value, key_in, query, Wq, bq, Wk, bk, Wv, bv, Wo, bo):
    import os
    from concourse.bass_utils import run_bass_kernel_spmd

    global LAST_EXEC_NS, LAST_RESULTS
    nc = _build_nc()
    in_maps = _host_prep(value, key_in, query, Wq, bq, Wk, bk, Wv, bv, Wo, bo)
    res = run_bass_kernel_spmd(nc, in_maps, list(range(NCORES)),
                               trace=bool(os.environ.get("BASS_TRACE")))
    LAST_EXEC_NS = res.exec_time_ns
    LAST_RESULTS = res
    bo = np.asarray(bo, np.float32)
    o = np.empty((BS, SL, D), np.float32)
    for b in range(BS):
        o[b] = res.results[2 * b]["out"] + res.results[2 * b + 1]["out"] + bo
    return o
